# revision 7
# baseline (speedup 1.0000x reference)
"""Multi-Head Latent Attention TRN2 kernel (8 NeuronCores, tensor-parallel over heads).

Strategy:
  - 16 heads sharded 2-per-core across 8 cores (tensor parallel).
  - The latent path is folded on the host: K = x @ (Wk_w @ Wc_w).T + (Wk_w @ Wc_b + Wk_b),
    same for V, so each core runs three D=2048-contraction projections for its
    256 head-columns, causal attention for its 2 heads, and a partial output
    projection; partials are summed on the host (allreduce-equivalent).
  - All activations are kept feature-major ("transposed") on device; x is
    transposed once on the host.
  - Softmax without max-subtraction (scores are O(1) by construction);
    denominator accumulated on DVE, reduced over partitions with a ones-matmul,
    reciprocal broadcast via a DRAM bounce, applied on the attention output.
  - Matmul dtype float32r (TF32-like, full PE rate at free-dim >= 256).
"""

import math

import numpy as np

import concourse.bass as bass
import concourse.tile as tile
from concourse import bacc, mybir
from concourse.bass import ds, ts
from concourse.bass_utils import run_bass_kernel_spmd

P = 128
B, S, D = 2, 2048, 2048
H, DL = 16, 512
HD = D // H              # 128
NCORES = 8
HLOC = H // NCORES       # 2 heads per core
COL = HLOC * HD          # 256 per-core head columns
TOK = B * S              # 4096
TT = 512                 # token tile (projections, q tiles)
DCH = D // P             # 16 contraction chunks
SCALE = 1.0 / math.sqrt(HD)

DT = mybir.dt.float32r   # matmul/SBUF dtype
DTP = mybir.dt.float32   # PSUM dtype
F32 = mybir.dt.float32

_CACHE = {}


def _build():
    nc = bacc.Bacc("TRN2", target_bir_lowering=False, debug=False)

    xT = nc.dram_tensor("xT", [D, TOK], DT, kind="ExternalInput").ap()
    wq = nc.dram_tensor("wq", [D, COL], DT, kind="ExternalInput").ap()
    wk = nc.dram_tensor("wk", [D, COL], DT, kind="ExternalInput").ap()
    wv = nc.dram_tensor("wv", [D, COL], DT, kind="ExternalInput").ap()
    wo = nc.dram_tensor("wo", [COL, D], DT, kind="ExternalInput").ap()
    bq = nc.dram_tensor("bq", [P, HLOC], F32, kind="ExternalInput").ap()
    bk = nc.dram_tensor("bk", [P, HLOC], F32, kind="ExternalInput").ap()
    bv = nc.dram_tensor("bv", [P, HLOC], F32, kind="ExternalInput").ap()
    masks = nc.dram_tensor("masks", [4, P, TT], DT, kind="ExternalInput").ap()
    identd = nc.dram_tensor("ident", [P, P], DT, kind="ExternalInput").ap()
    out = nc.dram_tensor("out", [TOK, D], F32, kind="ExternalOutput").ap()

    xT_r = xT.rearrange("(o p) t -> p o t", p=P)

    with tile.TileContext(nc) as tc:
        with (
            tc.tile_pool(name="const", bufs=1) as const,
            tc.tile_pool(name="wpool", bufs=1) as wpool,
            tc.tile_pool(name="xpool", bufs=3) as xpool,
            tc.tile_pool(name="actb", bufs=1) as actb,
            tc.tile_pool(name="expp", bufs=4) as expp,
            tc.tile_pool(name="daccp", bufs=2) as daccp,
            tc.tile_pool(name="stage", bufs=4) as stagep,
            tc.tile_pool(name="rp", bufs=2) as rp,
            tc.tile_pool(name="rbp", bufs=2) as rbp,
            tc.tile_pool(name="dram", bufs=4, space="DRAM") as dramp,
            tc.tile_pool(name="ps", bufs=8, space="PSUM") as psp,
        ):
            # constants
            ident = const.tile([P, P], DT)
            nc.sync.dma_start(ident, identd)
            ones = const.tile([P, 1], F32)
            nc.vector.memset(ones, 1.0)
            mask_sb = const.tile([P, 4, TT], DT)
            nc.sync.dma_start(mask_sb, masks.rearrange("m p q -> p m q"))
            bq_sb = const.tile([P, HLOC], F32, tag="bq")
            nc.sync.dma_start(bq_sb, bq)
            bk_sb = const.tile([P, HLOC], F32, tag="bk")
            nc.sync.dma_start(bk_sb, bk)
            bv_sb = const.tile([P, HLOC], F32, tag="bv")
            nc.sync.dma_start(bv_sb, bv)

            # weights resident, contraction-chunk layout
            wq_sb = wpool.tile([P, DCH, COL], DT, tag="wq")
            nc.sync.dma_start(wq_sb, wq.rearrange("(o p) c -> p o c", p=P))
            wk_sb = wpool.tile([P, DCH, COL], DT, tag="wk")
            nc.sync.dma_start(wk_sb, wk.rearrange("(o p) c -> p o c", p=P))
            wv_sb = wpool.tile([P, DCH, COL], DT, tag="wv")
            nc.sync.dma_start(wv_sb, wv.rearrange("(o p) c -> p o c", p=P))
            wo_sb = wpool.tile([P, HLOC, D], DT, tag="wo")
            nc.sync.dma_start(wo_sb, wo.rearrange("(c p) d -> p c d", p=P))

            for b in range(B):
                tok0 = b * S
                # per-batch activations (feature-major)
                qT_sb = actb.tile([P, HLOC, S], DT, tag="qT")
                kT_sb = actb.tile([P, HLOC, S], DT, tag="kT")
                v_sb = actb.tile([P, S // P, COL], DT, tag="v")
                o_sb = actb.tile([P, HLOC, S], DT, tag="o")

                # ---- Q/K/V projections ----
                for tt in range(S // TT):
                    ps_acc = [
                        psp.tile([P, TT], DTP, tag="ps", name=f"ps_acc{j}")
                        for j in range(6)
                    ]
                    for quarter in range(4):
                        x_sb = xpool.tile([P, 4, TT], DT, tag="x")
                        nc.sync.dma_start(
                            x_sb, xT_r[:, ds(quarter * 4, 4), ds(tok0 + tt * TT, TT)]
                        )
                        for kq in range(4):
                            k = quarter * 4 + kq
                            for j, w_sb in enumerate(
                                (wq_sb, wq_sb, wk_sb, wk_sb, wv_sb, wv_sb)
                            ):
                                cc = j % 2
                                nc.tensor.matmul(
                                    ps_acc[j],
                                    w_sb[:, k, ts(cc, P)],
                                    x_sb[:, kq, :],
                                    start=(k == 0),
                                    stop=(k == DCH - 1),
                                )
                    for cc in range(HLOC):
                        nc.vector.tensor_scalar_add(
                            qT_sb[:, cc, ts(tt, TT)], ps_acc[0 + cc], bq_sb[:, ds(cc, 1)]
                        )
                        nc.vector.tensor_scalar_add(
                            kT_sb[:, cc, ts(tt, TT)], ps_acc[2 + cc], bk_sb[:, ds(cc, 1)]
                        )
                        # V: evict transposed chunk then PE-transpose to token-major
                        vt_stage = stagep.tile([P, TT], DT, tag="stage")
                        nc.vector.tensor_scalar_add(
                            vt_stage, ps_acc[4 + cc], bv_sb[:, ds(cc, 1)]
                        )
                        for c4 in range(TT // P):
                            tp = psp.tile([P, P], DT, tag="ps")
                            nc.tensor.transpose(tp, vt_stage[:, ts(c4, P)], ident)
                            nc.vector.tensor_copy(
                                v_sb[:, tt * 4 + c4, ts(cc, P)], tp
                            )

                # ---- causal attention, 2 heads ----
                for h in range(HLOC):
                    for qt in range(S // TT):
                        o_ps = psp.tile([P, TT], DTP, tag="ps")
                        dacc = daccp.tile([P, TT], F32, tag="dacc")
                        nkt = (qt + 1) * (TT // P)
                        for kt in range(nkt):
                            s_ps = psp.tile([P, TT], DTP, tag="ps")
                            nc.tensor.matmul(
                                s_ps,
                                kT_sb[:, h, ts(kt, P)],
                                qT_sb[:, h, ts(qt, TT)],
                                start=True,
                                stop=True,
                            )
                            e_sb = expp.tile([P, TT], DT, tag="exp")
                            nc.scalar.activation(
                                e_sb, s_ps, mybir.ActivationFunctionType.Exp, scale=SCALE
                            )
                            m = kt - qt * (TT // P)
                            if 0 <= m < 4:
                                nc.vector.tensor_tensor(
                                    e_sb, e_sb, mask_sb[:, m, :], mybir.AluOpType.mult
                                )
                            if kt == 0:
                                nc.vector.tensor_copy(dacc, e_sb)
                            else:
                                nc.vector.tensor_add(dacc, dacc, e_sb)
                            nc.tensor.matmul(
                                o_ps,
                                v_sb[:, kt, ts(h, P)],
                                e_sb,
                                start=(kt == 0),
                                stop=(kt == nkt - 1),
                            )
                        # denominator: partition-reduce, reciprocal, broadcast
                        dr_ps = psp.tile([P, TT], DTP, tag="ps")
                        nc.tensor.matmul(dr_ps[:1, :], ones, dacc, start=True, stop=True)
                        r_seg = rp.tile([1, TT], F32, tag="r")
                        nc.vector.reciprocal(r_seg, dr_ps[:1, :])
                        r_dram = dramp.tile([1, TT], F32)
                        nc.sync.dma_start(r_dram, r_seg)
                        rb_sb = rbp.tile([P, TT], F32, tag="rb")
                        nc.sync.dma_start(rb_sb, r_dram.to_broadcast((P, TT)))
                        nc.vector.tensor_tensor(
                            o_sb[:, h, ts(qt, TT)], o_ps, rb_sb, mybir.AluOpType.mult
                        )

                # ---- partial output projection (token-major) ----
                for tch in range(S // P):
                    for nt in range(D // TT):
                        p_ps = psp.tile([P, TT], DTP, tag="ps")
                        for h in range(HLOC):
                            nc.tensor.matmul(
                                p_ps,
                                o_sb[:, h, ts(tch, P)],
                                wo_sb[:, h, ts(nt, TT)],
                                start=(h == 0),
                                stop=(h == HLOC - 1),
                            )
                        o_stage = stagep.tile([P, TT], F32, tag="ostage")
                        nc.vector.tensor_copy(o_stage, p_ps)
                        nc.sync.dma_start(
                            out[ds(tok0 + tch * P, P), ts(nt, TT)], o_stage
                        )
    nc.compile()
    return nc


def _prep_inputs(x, Wq_w, Wq_b, Wc_w, Wc_b, Wk_w, Wk_b, Wv_w, Wv_b, Wo_w, Wo_b):
    f32 = np.float32
    x = np.ascontiguousarray(np.asarray(x, f32).reshape(TOK, D))
    xT = np.ascontiguousarray(x.T)
    Wk_eff = np.asarray(Wk_w, f32) @ np.asarray(Wc_w, f32)     # [D, D]
    Wv_eff = np.asarray(Wv_w, f32) @ np.asarray(Wc_w, f32)
    bk_eff = np.asarray(Wk_w, f32) @ np.asarray(Wc_b, f32) + np.asarray(Wk_b, f32)
    bv_eff = np.asarray(Wv_w, f32) @ np.asarray(Wc_b, f32) + np.asarray(Wv_b, f32)

    m = (np.arange(P)[None, :, None] + P * np.arange(4)[:, None, None]
         <= np.arange(TT)[None, None, :]).astype(f32)          # [4, 128, 512]

    in_maps = []
    for c in range(NCORES):
        cols = slice(c * COL, (c + 1) * COL)
        in_maps.append({
            "xT": xT,
            "wq": np.ascontiguousarray(np.asarray(Wq_w, f32)[cols, :].T),
            "wk": np.ascontiguousarray(Wk_eff[cols, :].T),
            "wv": np.ascontiguousarray(Wv_eff[cols, :].T),
            "wo": np.ascontiguousarray(np.asarray(Wo_w, f32)[:, cols].T),
            "bq": np.ascontiguousarray(np.asarray(Wq_b, f32)[cols].reshape(HLOC, P).T),
            "bk": np.ascontiguousarray(bk_eff[cols].reshape(HLOC, P).T),
            "bv": np.ascontiguousarray(bv_eff[cols].reshape(HLOC, P).T),
            "masks": m,
            "ident": np.eye(P, dtype=f32),
        })
    return in_maps


def kernel(**inputs):
    if "nc" not in _CACHE:
        _CACHE["nc"] = _build()
    nc = _CACHE["nc"]
    in_maps = _prep_inputs(**inputs)
    res = run_bass_kernel_spmd(nc, in_maps, core_ids=list(range(NCORES)))
    acc = res.results[0]["out"].astype(np.float32)
    for c in range(1, NCORES):
        acc = acc + res.results[c]["out"]
    acc = acc + np.asarray(inputs["Wo_b"], np.float32)[None, :]
    return acc.reshape(B, S, D)


# revision 20
# speedup vs baseline: 2.0964x; 2.0964x over previous
"""Multi-Head Latent Attention TRN2 kernel (8 NeuronCores, tensor-parallel over heads).

Strategy:
  - 16 heads sharded 2-per-core across 8 cores (tensor parallel).
  - Latent path folded on the host: K = x @ (Wk_w @ Wc_w).T + (Wk_w @ Wc_b + Wk_b),
    same for V. Each core: three D=2048-contraction projections for its 256
    head-columns, causal attention for its 2 heads, partial output projection;
    partials summed on the host (allreduce-equivalent).
  - Activations feature-major ("transposed") on device; x transposed on host.
  - Softmax without max-subtraction (scores are O(1) by construction);
    denominator accumulated on DVE, partition-reduced with a ones-matmul,
    reciprocal broadcast via a DRAM bounce, applied to the attention output.
    Causal masking multiplies exp tiles on GPSIMD.
  - Matmul dtype float32r (TF32-like; measured ~125 ns per 128x128x512 MM).
"""

import contextlib
import math

import numpy as np

import concourse.bass as bass
import concourse.tile as tile
from concourse import bacc, mybir
from concourse.bass import ds, ts
from concourse.bass_utils import run_bass_kernel_spmd

P = 128
B, S, D = 2, 2048, 2048
H, DL = 16, 512
HD = D // H              # 128
NCORES = 8
HLOC = H // NCORES       # 2 heads per core
COL = HLOC * HD          # 256 per-core head columns
TOK = B * S              # 4096
TT = 512                 # token tile (projections, q tiles)
DCH = D // P             # 16 contraction chunks
SCALE = 1.0 / math.sqrt(HD)

DT = mybir.dt.float32r   # matmul/SBUF dtype
DTP = mybir.dt.float32   # PSUM dtype
F32 = mybir.dt.float32

_CACHE = {}


def _build(repeat=1, phases=(1, 1, 1), interleave=False, expp_bufs=5, mask_engine="mm", denom="pe", early_evict=True, x_bf16=True, out_bf16=True):
    nc = bacc.Bacc("TRN2", target_bir_lowering=False, debug=False)

    xdt = mybir.dt.bfloat16 if x_bf16 else DT
    odt = mybir.dt.bfloat16 if out_bf16 else F32
    xT = nc.dram_tensor("xT", [D, TOK], xdt, kind="ExternalInput").ap()
    wq = nc.dram_tensor("wq", [D, COL], DT, kind="ExternalInput").ap()
    wk = nc.dram_tensor("wk", [D, COL], DT, kind="ExternalInput").ap()
    wv = nc.dram_tensor("wv", [D, COL], DT, kind="ExternalInput").ap()
    wo = nc.dram_tensor("wo", [COL, D], DT, kind="ExternalInput").ap()
    bq = nc.dram_tensor("bq", [P, HLOC], F32, kind="ExternalInput").ap()
    bk = nc.dram_tensor("bk", [P, HLOC], F32, kind="ExternalInput").ap()
    bv = nc.dram_tensor("bv", [P, HLOC], F32, kind="ExternalInput").ap()
    masks = nc.dram_tensor("masks", [4, P, TT], DT, kind="ExternalInput").ap()
    identd = nc.dram_tensor("ident", [P, P], DT, kind="ExternalInput").ap()
    onesd = nc.dram_tensor("ones", [P, 1], DT, kind="ExternalInput").ap()
    out = nc.dram_tensor("out", [TOK, D], odt, kind="ExternalOutput").ap()

    xT_r = xT.rearrange("(o p) t -> p o t", p=P)

    with tile.TileContext(nc) as tc:
        with (
            tc.tile_pool(name="const", bufs=1) as const,
            tc.tile_pool(name="wpool", bufs=1) as wpool,
            tc.tile_pool(name="xpool", bufs=4) as xpool,
            tc.tile_pool(name="actb", bufs=1) as actb,
            tc.tile_pool(name="expp", bufs=expp_bufs) as expp,
            tc.tile_pool(name="daccp", bufs=2) as daccp,
            tc.tile_pool(name="stage", bufs=4) as stagep,
            tc.tile_pool(name="rp", bufs=2) as rp,
            tc.tile_pool(name="rbp", bufs=2) as rbp,
            tc.tile_pool(name="dram", bufs=4, space="DRAM") as dramp,
            tc.tile_pool(name="ps", bufs=8, space="PSUM") as psp,
        ):
            ident = const.tile([P, P], DT)
            nc.sync.dma_start(ident, identd)
            ones = const.tile([P, 1], DT)
            nc.sync.dma_start(ones, onesd)
            bq_sb = const.tile([P, HLOC], F32, tag="bq")
            nc.sync.dma_start(bq_sb, bq)
            bk_sb = const.tile([P, HLOC], F32, tag="bk")
            nc.sync.dma_start(bk_sb, bk)
            bv_sb = const.tile([P, HLOC], F32, tag="bv")
            nc.sync.dma_start(bv_sb, bv)

            wq_sb = wpool.tile([P, DCH, COL], DT, tag="wq")
            nc.sync.dma_start(wq_sb, wq.rearrange("(o p) c -> p o c", p=P))
            wk_sb = wpool.tile([P, DCH, COL], DT, tag="wk")
            nc.sync.dma_start(wk_sb, wk.rearrange("(o p) c -> p o c", p=P))
            wv_sb = wpool.tile([P, DCH, COL], DT, tag="wv")
            nc.sync.dma_start(wv_sb, wv.rearrange("(o p) c -> p o c", p=P))
            mask_sb = const.tile([P, 4, TT], DT)
            nc.sync.dma_start(mask_sb, masks.rearrange("m p q -> p m q"))
            wo_sb = wpool.tile([P, HLOC, D], DT, tag="wo")
            nc.sync.dma_start(wo_sb, wo.rearrange("(c p) d -> p c d", p=P))

            state = {"flip": 0}

            def emit_qkv(tok0, tt, qT_sb, kT_sb, v_sb):
                ps_acc = [
                    psp.tile([P, TT], DTP, tag="ps", name=f"ps_acc{j}")
                    for j in range(6)
                ]
                for quarter in range(4):
                    x_sb = xpool.tile([P, 4, TT], DT, tag="x", name="x_sb")
                    (nc.gpsimd if x_bf16 else nc.sync).dma_start(
                        x_sb, xT_r[:, ds(quarter * 4, 4), ds(tok0 + tt * TT, TT)]
                    )
                    for kq in range(4):
                        k = quarter * 4 + kq
                        for j, w_sb in enumerate(
                            (wq_sb, wq_sb, wk_sb, wk_sb, wv_sb, wv_sb)
                        ):
                            cc = j % 2
                            nc.tensor.matmul(
                                ps_acc[j],
                                w_sb[:, k, ts(cc, P)],
                                x_sb[:, kq, :],
                                start=(k == 0),
                                stop=(k == DCH - 1),
                            )
                vt_stages = []
                for cc in range(HLOC):
                    nc.vector.tensor_scalar_add(
                        qT_sb[:, cc, ts(tt, TT)], ps_acc[0 + cc], bq_sb[:, ds(cc, 1)]
                    )
                    nc.vector.tensor_scalar_add(
                        kT_sb[:, cc, ts(tt, TT)], ps_acc[2 + cc], bk_sb[:, ds(cc, 1)]
                    )
                    vt_stage = stagep.tile([P, TT], DT, tag="stage", name="vt_stage")
                    nc.vector.tensor_scalar_add(
                        vt_stage, ps_acc[4 + cc], bv_sb[:, ds(cc, 1)]
                    )
                    vt_stages.append((cc, vt_stage))

                def flush(cc, vt_stage, tt=tt):
                    for c4 in range(TT // P):
                        tp = psp.tile([P, P], DT, tag="ps", name="tp")
                        nc.tensor.transpose(tp, vt_stage[:, ts(c4, P)], ident)
                        nc.vector.tensor_copy(v_sb[:, tt * 4 + c4, ts(cc, P)], tp)

                return [lambda cc=cc, v=v: flush(cc, v) for cc, v in vt_stages]

            def emit_attn(qt, qT_sb, kT_sb, v_sb, o_sb, L=3):
                nkt = (qt + 1) * (TT // P)
                o_ps = {}
                d_ps = {}
                e_tiles = {}
                for h in range(HLOC):
                    o_ps[h] = psp.tile([P, TT], DTP, tag="ps", name="o_ps")
                    d_ps[h] = psp.tile([P, TT], DTP, tag="ps", name="d_ps")

                def emit_score(h, kt):
                    s_ps = psp.tile([P, TT], DTP, tag="ps", name="s_ps")
                    m = kt - qt * (TT // P)
                    diag = 0 <= m < 4
                    if diag:
                        nc.tensor.matmul(
                            s_ps, ident, mask_sb[:, m, :], start=True, stop=False
                        )
                    nc.tensor.matmul(
                        s_ps,
                        kT_sb[:, h, ts(kt, P)],
                        qT_sb[:, h, ts(qt, TT)],
                        start=not diag,
                        stop=True,
                    )
                    e_sb = expp.tile([P, TT], DT, tag="exp", name="e_sb")
                    nc.scalar.activation(
                        e_sb, s_ps, mybir.ActivationFunctionType.Exp, scale=SCALE
                    )
                    e_tiles[(h, kt)] = e_sb

                def emit_consume(h, kt):
                    e_sb = e_tiles.pop((h, kt))
                    nc.tensor.matmul(
                        d_ps[h][:1, :], ones, e_sb,
                        start=(kt == 0), stop=(kt == nkt - 1),
                    )
                    nc.tensor.matmul(
                        o_ps[h],
                        v_sb[:, kt, ts(h, P)],
                        e_sb,
                        start=(kt == 0),
                        stop=(kt == nkt - 1),
                    )

                # software pipeline: scores run L blocks ahead of consumers
                for kt in range(min(L, nkt)):
                    for h in range(HLOC):
                        emit_score(h, kt)
                for kt in range(nkt):
                    for h in range(HLOC):
                        emit_consume(h, kt)
                        if kt + L < nkt:
                            emit_score(h, kt + L)

                for h in range(HLOC):
                    r_seg = rp.tile([1, TT], F32, tag="r", name="r_seg")
                    nc.vector.reciprocal(r_seg, d_ps[h][:1, :])
                    r_dram = dramp.tile([1, TT], F32, name="r_dram")
                    nc.sync.dma_start(r_dram, r_seg)
                    rb_sb = rbp.tile([P, TT], F32, tag="rb", name="rb_sb")
                    nc.sync.dma_start(rb_sb, r_dram.to_broadcast((P, TT)))
                    nc.vector.tensor_copy(o_sb[:, h, ts(qt, TT)], o_ps[h])
                    nc.vector.tensor_tensor(
                        o_sb[:, h, ts(qt, TT)], o_sb[:, h, ts(qt, TT)], rb_sb,
                        mybir.AluOpType.mult
                    )

            def emit_proj(tok0, qt, o_sb):
                for tc4 in range(TT // P):
                    tch = qt * 4 + tc4
                    for nt in range(D // TT):
                        p_ps = psp.tile([P, TT], DTP, tag="ps", name="p_ps")
                        for h in range(HLOC):
                            nc.tensor.matmul(
                                p_ps,
                                o_sb[:, h, ts(tch, P)],
                                wo_sb[:, h, ts(nt, TT)],
                                start=(h == 0),
                                stop=(h == HLOC - 1),
                            )
                        o_stage = stagep.tile([P, TT], odt, tag="ostage", name="o_stage")
                        if state["flip"] % 2 == 0:
                            nc.vector.tensor_copy(o_stage, p_ps)
                        else:
                            nc.scalar.activation(
                                o_stage, p_ps, mybir.ActivationFunctionType.Copy
                            )
                        state["flip"] += 1
                        nc.sync.dma_start(
                            out[ds(tok0 + tch * P, P), ts(nt, TT)], o_stage
                        )

            rep_ctx = tc.For_i(0, repeat, 1) if repeat > 1 else contextlib.nullcontext()
            with rep_ctx:
                for b in range(B):
                    tok0 = b * S
                    qT_sb = actb.tile([P, HLOC, S], DT, tag="qT", name="qT_sb")
                    kT_sb = actb.tile([P, HLOC, S], DT, tag="kT", name="kT_sb")
                    v_sb = actb.tile([P, S // P, COL], DT, tag="v", name="v_sb")
                    o_sb = actb.tile([P, HLOC, S], DT, tag="o", name="o_sb")

                    pending_tp = []
                    if phases[0]:
                        for tt in range(S // TT):
                            tps = emit_qkv(tok0, tt, qT_sb, kT_sb, v_sb)
                            for f in pending_tp:
                                f()
                            pending_tp = tps
                        for f in pending_tp:
                            f()
                    prev_qt = None
                    for qt in range(S // TT):
                        if phases[1]:
                            emit_attn(qt, qT_sb, kT_sb, v_sb, o_sb)
                        if phases[2] and prev_qt is not None:
                            emit_proj(tok0, prev_qt, o_sb)
                        prev_qt = qt
                    if phases[2] and prev_qt is not None:
                        emit_proj(tok0, prev_qt, o_sb)
    nc.compile()
    return nc


def _prep_inputs(x, Wq_w, Wq_b, Wc_w, Wc_b, Wk_w, Wk_b, Wv_w, Wv_b, Wo_w, Wo_b, x_bf16=True, mask_mode="mm"):
    import ml_dtypes
    f32 = np.float32
    x = np.ascontiguousarray(np.asarray(x, f32).reshape(TOK, D))
    xT = np.ascontiguousarray(x.T)
    Wk_eff = np.asarray(Wk_w, f32) @ np.asarray(Wc_w, f32)     # [D, D]
    Wv_eff = np.asarray(Wv_w, f32) @ np.asarray(Wc_w, f32)
    bk_eff = np.asarray(Wk_w, f32) @ np.asarray(Wc_b, f32) + np.asarray(Wk_b, f32)
    bv_eff = np.asarray(Wv_w, f32) @ np.asarray(Wc_b, f32) + np.asarray(Wv_b, f32)

    keep = (np.arange(P)[None, :, None] + P * np.arange(4)[:, None, None]
            <= np.arange(TT)[None, None, :])                   # [4, 128, 512]
    if mask_mode == "mm":
        m = np.where(keep, 0.0, -340.0).astype(f32)
    else:
        m = keep.astype(f32)

    in_maps = []
    for c in range(NCORES):
        cols = slice(c * COL, (c + 1) * COL)
        in_maps.append({
            "xT": xT.astype(ml_dtypes.bfloat16) if x_bf16 else xT,
            "wq": np.ascontiguousarray(np.asarray(Wq_w, f32)[cols, :].T),
            "wk": np.ascontiguousarray(Wk_eff[cols, :].T),
            "wv": np.ascontiguousarray(Wv_eff[cols, :].T),
            "wo": np.ascontiguousarray(np.asarray(Wo_w, f32)[:, cols].T),
            "bq": np.ascontiguousarray(np.asarray(Wq_b, f32)[cols].reshape(HLOC, P).T),
            "bk": np.ascontiguousarray(bk_eff[cols].reshape(HLOC, P).T),
            "bv": np.ascontiguousarray(bv_eff[cols].reshape(HLOC, P).T),
            "masks": m,
            "ident": np.eye(P, dtype=f32),
            "ones": np.ones((P, 1), f32),
        })
    return in_maps


def kernel(**inputs):
    if "nc" not in _CACHE:
        _CACHE["nc"] = _build()
    nc = _CACHE["nc"]
    in_maps = _prep_inputs(**inputs)
    res = run_bass_kernel_spmd(nc, in_maps, core_ids=list(range(NCORES)))
    acc = res.results[0]["out"].astype(np.float32)
    for c in range(1, NCORES):
        acc = acc + res.results[c]["out"]
    acc = acc + np.asarray(inputs["Wo_b"], np.float32)[None, :]
    return acc.reshape(B, S, D)


# revision 29
# speedup vs baseline: 2.1859x; 1.0427x over previous
"""Multi-Head Latent Attention TRN2 kernel (8 NeuronCores, tensor-parallel over heads).

Strategy:
  - 16 heads sharded 2-per-core across 8 cores (tensor parallel).
  - Latent path folded on the host: K = x @ (Wk_w @ Wc_w).T + (Wk_w @ Wc_b + Wk_b),
    same for V. Each core: three D=2048-contraction projections for its 256
    head-columns, causal attention for its 2 heads, partial output projection;
    partials summed on the host (allreduce-equivalent).
  - Activations feature-major ("transposed") on device; x transposed on host.
  - Softmax without max-subtraction (scores are O(1) by construction);
    denominator accumulated on DVE, partition-reduced with a ones-matmul,
    reciprocal broadcast via a DRAM bounce, applied to the attention output.
    Causal masking multiplies exp tiles on GPSIMD.
  - Matmul dtype float32r (TF32-like; measured ~125 ns per 128x128x512 MM).
"""

import contextlib
import math

import numpy as np

import concourse.bass as bass
import concourse.tile as tile
from concourse import bacc, mybir
from concourse.bass import ds, ts
from concourse.bass_utils import run_bass_kernel_spmd

P = 128
B, S, D = 2, 2048, 2048
H, DL = 16, 512
HD = D // H              # 128
NCORES = 8
HLOC = H // NCORES       # 2 heads per core
COL = HLOC * HD          # 256 per-core head columns
TOK = B * S              # 4096
TT = 512                 # token tile (projections, q tiles)
DCH = D // P             # 16 contraction chunks
SCALE = 1.0 / math.sqrt(HD)

DT = mybir.dt.float32r   # matmul/SBUF dtype
DTP = mybir.dt.float32   # PSUM dtype
F32 = mybir.dt.float32

_CACHE = {}


def _build(repeat=1, phases=(1, 1, 1), interleave=False, ATTN_L=5, mask_engine="mm", denom="dve", early_evict=True, x_bf16=True, out_bf16=True, use_bias=False):
    nc = bacc.Bacc("TRN2", target_bir_lowering=False, debug=False)

    xdt = mybir.dt.bfloat16 if x_bf16 else DT
    odt = mybir.dt.bfloat16 if out_bf16 else F32
    xT = nc.dram_tensor("xT", [B, S // TT, 4, P, 4, TT], xdt, kind="ExternalInput").ap()
    wq = nc.dram_tensor("wq", [D, COL], DT, kind="ExternalInput").ap()
    wk = nc.dram_tensor("wk", [D, COL], DT, kind="ExternalInput").ap()
    wv = nc.dram_tensor("wv", [D, COL], DT, kind="ExternalInput").ap()
    wo = nc.dram_tensor("wo", [COL, D], DT, kind="ExternalInput").ap()
    bq = nc.dram_tensor("bq", [P, HLOC], F32, kind="ExternalInput").ap()
    bk = nc.dram_tensor("bk", [P, HLOC], F32, kind="ExternalInput").ap()
    bv = nc.dram_tensor("bv", [P, HLOC], F32, kind="ExternalInput").ap()
    masks = nc.dram_tensor("masks", [4, P, TT], DT, kind="ExternalInput").ap()
    identd = nc.dram_tensor("ident", [P, P], DT, kind="ExternalInput").ap()
    onesd = nc.dram_tensor("ones", [P, 1], DT, kind="ExternalInput").ap()
    out = nc.dram_tensor("out", [TOK, D], odt, kind="ExternalOutput").ap()

    with tile.TileContext(nc) as tc:
        with (
            tc.tile_pool(name="const", bufs=1) as const,
            tc.tile_pool(name="wpool", bufs=1) as wpool,
            tc.tile_pool(name="xpool", bufs=4) as xpool,
            tc.tile_pool(name="actb", bufs=1) as actb,
            tc.tile_pool(name="expp", bufs=8) as expp,
            tc.tile_pool(name="daccp", bufs=(2 if denom != "pe" else 1)) as daccp,
            tc.tile_pool(name="stage", bufs=4) as stagep,
            tc.tile_pool(name="rp", bufs=2) as rp,
            tc.tile_pool(name="rbp", bufs=2) as rbp,
            tc.tile_pool(name="dram", bufs=4, space="DRAM") as dramp,
            tc.tile_pool(name="ps", bufs=8, space="PSUM") as psp,
        ):
            ident = const.tile([P, P], DT)
            nc.sync.dma_start(ident, identd)
            ones = const.tile([P, 1], DT)
            nc.sync.dma_start(ones, onesd)
            bq_sb = const.tile([P, HLOC], F32, tag="bq")
            nc.sync.dma_start(bq_sb, bq)
            bk_sb = const.tile([P, HLOC], F32, tag="bk")
            nc.sync.dma_start(bk_sb, bk)
            bv_sb = const.tile([P, HLOC], F32, tag="bv")
            nc.sync.dma_start(bv_sb, bv)

            wq_sb = wpool.tile([P, DCH, COL], DT, tag="wq")
            nc.sync.dma_start(wq_sb, wq.rearrange("(o p) c -> p o c", p=P))
            wk_sb = wpool.tile([P, DCH, COL], DT, tag="wk")
            nc.sync.dma_start(wk_sb, wk.rearrange("(o p) c -> p o c", p=P))
            wv_sb = wpool.tile([P, DCH, COL], DT, tag="wv")
            nc.sync.dma_start(wv_sb, wv.rearrange("(o p) c -> p o c", p=P))
            mask_sb = const.tile([P, 4, TT], DT)
            nc.sync.dma_start(mask_sb, masks.rearrange("m p q -> p m q"))
            wo_sb = wpool.tile([P, HLOC, D], DT, tag="wo")
            nc.sync.dma_start(wo_sb, wo.rearrange("(c p) d -> p c d", p=P))

            state = {"flip": 0}

            def emit_qkv(b, tok0, tt, qT_sb, kT_sb, v_sb):
                ps_acc = [
                    psp.tile([P, TT], DTP, tag="ps", name=f"ps_acc{j}")
                    for j in range(6)
                ]
                for quarter in range(4):
                    x_sb = xpool.tile([P, 4, TT], DT, tag="x", name="x_sb")
                    (nc.gpsimd if x_bf16 else nc.sync).dma_start(
                        x_sb, xT[b, tt, quarter]
                    )
                    for kq in range(4):
                        k = quarter * 4 + kq
                        for j, w_sb in enumerate(
                            (wq_sb, wq_sb, wk_sb, wk_sb, wv_sb, wv_sb)
                        ):
                            cc = j % 2
                            nc.tensor.matmul(
                                ps_acc[j],
                                w_sb[:, k, ts(cc, P)],
                                x_sb[:, kq, :],
                                start=(k == 0),
                                stop=(k == DCH - 1),
                            )
                vt_stages = []
                for cc in range(HLOC):
                    vt_stage = stagep.tile([P, TT], DT, tag="stage", name="vt_stage")
                    if use_bias:
                        nc.vector.tensor_scalar_add(
                            qT_sb[:, cc, ts(tt, TT)], ps_acc[0 + cc], bq_sb[:, ds(cc, 1)]
                        )
                        nc.vector.tensor_scalar_add(
                            kT_sb[:, cc, ts(tt, TT)], ps_acc[2 + cc], bk_sb[:, ds(cc, 1)]
                        )
                        nc.vector.tensor_scalar_add(
                            vt_stage, ps_acc[4 + cc], bv_sb[:, ds(cc, 1)]
                        )
                    else:
                        nc.vector.tensor_copy(qT_sb[:, cc, ts(tt, TT)], ps_acc[0 + cc])
                        nc.vector.tensor_copy(kT_sb[:, cc, ts(tt, TT)], ps_acc[2 + cc])
                        nc.vector.tensor_copy(vt_stage, ps_acc[4 + cc])
                    vt_stages.append((cc, vt_stage))

                def flush(cc, vt_stage, tt=tt):
                    for c4 in range(TT // P):
                        tp = psp.tile([P, P], DT, tag="ps", name="tp")
                        nc.tensor.transpose(tp, vt_stage[:, ts(c4, P)], ident)
                        nc.vector.tensor_copy(v_sb[:, tt * 4 + c4, ts(cc, P)], tp)

                return [lambda cc=cc, v=v: flush(cc, v) for cc, v in vt_stages]

            def emit_attn(qt, qT_sb, kT_sb, v_sb, o_sb, L=ATTN_L):
                nkt = (qt + 1) * (TT // P)
                o_ps = {}
                d_ps = {}
                e_tiles = {}
                for h in range(HLOC):
                    o_ps[h] = psp.tile([P, TT], DTP, tag="ps", name="o_ps")
                    d_ps[h] = psp.tile([P, TT], DTP, tag="ps", name="d_ps")[ds(0, 1), :]

                def emit_score(h, kt):
                    s_ps = psp.tile([P, TT], DTP, tag="ps", name="s_ps")
                    m = kt - qt * (TT // P)
                    diag = 0 <= m < 4
                    if diag:
                        nc.tensor.matmul(
                            s_ps, ident, mask_sb[:, m, :], start=True, stop=False
                        )
                    nc.tensor.matmul(
                        s_ps,
                        kT_sb[:, h, ts(kt, P)],
                        qT_sb[:, h, ts(qt, TT)],
                        start=not diag,
                        stop=True,
                    )
                    e_sb = expp.tile([P, TT], DT, tag="exp", name="e_sb")
                    nc.scalar.activation(
                        e_sb, s_ps, mybir.ActivationFunctionType.Exp, scale=SCALE
                    )
                    e_tiles[(h, kt)] = e_sb

                def emit_consume(h, kt):
                    e_sb = e_tiles.pop((h, kt))
                    nc.tensor.matmul(
                        d_ps[h], ones, e_sb,
                        start=(kt == 0), stop=(kt == nkt - 1),
                    )
                    nc.tensor.matmul(
                        o_ps[h],
                        v_sb[:, kt, ts(h, P)],
                        e_sb,
                        start=(kt == 0),
                        stop=(kt == nkt - 1),
                    )

                # software pipeline: scores run L blocks ahead of consumers
                for kt in range(min(L, nkt)):
                    for h in range(HLOC):
                        emit_score(h, kt)
                for kt in range(nkt):
                    for h in range(HLOC):
                        emit_consume(h, kt)
                        if kt + L < nkt:
                            emit_score(h, kt + L)

                for h in range(HLOC):
                    r_seg = rp.tile([1, TT], F32, tag="r", name="r_seg")
                    nc.vector.reciprocal(r_seg, d_ps[h])
                    r_dram = dramp.tile([1, TT], F32, name="r_dram")
                    nc.sync.dma_start(r_dram, r_seg)
                    rb_sb = rbp.tile([P, TT], F32, tag="rb", name="rb_sb")
                    nc.sync.dma_start(rb_sb, r_dram.to_broadcast((P, TT)))
                    nc.vector.tensor_copy(o_sb[:, h, ts(qt, TT)], o_ps[h])
                    nc.vector.tensor_tensor(
                        o_sb[:, h, ts(qt, TT)], o_sb[:, h, ts(qt, TT)], rb_sb,
                        mybir.AluOpType.mult
                    )

            def emit_proj(tok0, qt, o_sb):
                for tc4 in range(TT // P):
                    tch = qt * 4 + tc4
                    for nt in range(D // TT):
                        p_ps = psp.tile([P, TT], DTP, tag="ps", name="p_ps")
                        for h in range(HLOC):
                            nc.tensor.matmul(
                                p_ps,
                                o_sb[:, h, ts(tch, P)],
                                wo_sb[:, h, ts(nt, TT)],
                                start=(h == 0),
                                stop=(h == HLOC - 1),
                            )
                        o_stage = stagep.tile([P, TT], odt, tag="ostage", name="o_stage")
                        if state["flip"] % 4 != 3:
                            nc.vector.tensor_copy(o_stage, p_ps)
                        else:
                            nc.scalar.activation(
                                o_stage, p_ps, mybir.ActivationFunctionType.Copy
                            )
                        state["flip"] += 1
                        nc.sync.dma_start(
                            out[ds(tok0 + tch * P, P), ts(nt, TT)], o_stage
                        )

            rep_ctx = tc.For_i(0, repeat, 1) if repeat > 1 else contextlib.nullcontext()
            with rep_ctx:
                for b in range(B):
                    tok0 = b * S
                    qT_sb = actb.tile([P, HLOC, S], DT, tag="qT", name="qT_sb")
                    kT_sb = actb.tile([P, HLOC, S], DT, tag="kT", name="kT_sb")
                    v_sb = actb.tile([P, S // P, COL], DT, tag="v", name="v_sb")
                    o_sb = actb.tile([P, HLOC, S], DT, tag="o", name="o_sb")

                    pending_tp = []
                    if phases[0]:
                        for tt in range(S // TT):
                            tps = emit_qkv(b, tok0, tt, qT_sb, kT_sb, v_sb)
                            for f in pending_tp:
                                f()
                            pending_tp = tps
                        for f in pending_tp:
                            f()
                    prev_qt = None
                    for qt in range(S // TT):
                        if phases[1]:
                            emit_attn(qt, qT_sb, kT_sb, v_sb, o_sb)
                        if phases[2] and prev_qt is not None:
                            emit_proj(tok0, prev_qt, o_sb)
                        prev_qt = qt
                    if phases[2] and prev_qt is not None:
                        emit_proj(tok0, prev_qt, o_sb)
    nc.compile()
    return nc


def _prep_inputs(x, Wq_w, Wq_b, Wc_w, Wc_b, Wk_w, Wk_b, Wv_w, Wv_b, Wo_w, Wo_b, x_bf16=True, mask_mode="mm"):
    import ml_dtypes
    f32 = np.float32
    x = np.ascontiguousarray(np.asarray(x, f32).reshape(TOK, D))
    xT = np.ascontiguousarray(x.T)
    # pre-arranged contiguous blocks: [B, S//TT, 2, 128, 8, TT]
    # xprep[b, tt, h, p, i, t] = xT[(h*8+i)*128 + p, b*S + tt*TT + t]
    xp = xT.reshape(4, 4, P, B, S // TT, TT)         # [q, i, p, b, tt, t]
    xprep = np.ascontiguousarray(xp.transpose(3, 4, 0, 2, 1, 5))
    if x_bf16:
        xprep = xprep.astype(ml_dtypes.bfloat16)
    else:
        xprep = xprep.astype(f32)
    Wk_eff = np.asarray(Wk_w, f32) @ np.asarray(Wc_w, f32)     # [D, D]
    Wv_eff = np.asarray(Wv_w, f32) @ np.asarray(Wc_w, f32)
    bk_eff = np.asarray(Wk_w, f32) @ np.asarray(Wc_b, f32) + np.asarray(Wk_b, f32)
    bv_eff = np.asarray(Wv_w, f32) @ np.asarray(Wc_b, f32) + np.asarray(Wv_b, f32)

    keep = (np.arange(P)[None, :, None] + P * np.arange(4)[:, None, None]
            <= np.arange(TT)[None, None, :])                   # [4, 128, 512]
    if mask_mode == "mm":
        m = np.where(keep, 0.0, -340.0).astype(f32)
    else:
        m = keep.astype(f32)

    in_maps = []
    for c in range(NCORES):
        cols = slice(c * COL, (c + 1) * COL)
        in_maps.append({
            "xT": xprep,
            "wq": np.ascontiguousarray(np.asarray(Wq_w, f32)[cols, :].T),
            "wk": np.ascontiguousarray(Wk_eff[cols, :].T),
            "wv": np.ascontiguousarray(Wv_eff[cols, :].T),
            "wo": np.ascontiguousarray(np.asarray(Wo_w, f32)[:, cols].T),
            "bq": np.ascontiguousarray(np.asarray(Wq_b, f32)[cols].reshape(HLOC, P).T),
            "bk": np.ascontiguousarray(bk_eff[cols].reshape(HLOC, P).T),
            "bv": np.ascontiguousarray(bv_eff[cols].reshape(HLOC, P).T),
            "masks": m,
            "ident": np.eye(P, dtype=f32),
            "ones": np.ones((P, 1), f32),
        })
    return in_maps


def kernel(**inputs):
    use_bias = any(
        np.any(np.asarray(inputs[k])) for k in ("Wq_b", "Wc_b", "Wk_b", "Wv_b")
    )
    key = ("nc", bool(use_bias))
    if key not in _CACHE:
        _CACHE[key] = _build(use_bias=use_bias)
    nc = _CACHE[key]
    in_maps = _prep_inputs(**inputs)
    res = run_bass_kernel_spmd(nc, in_maps, core_ids=list(range(NCORES)))
    acc = res.results[0]["out"].astype(np.float32)
    for c in range(1, NCORES):
        acc = acc + res.results[c]["out"]
    acc = acc + np.asarray(inputs["Wo_b"], np.float32)[None, :]
    return acc.reshape(B, S, D)


# revision 33
# speedup vs baseline: 2.2274x; 1.0190x over previous
"""Multi-Head Latent Attention TRN2 kernel (8 NeuronCores, tensor-parallel over heads).

Strategy:
  - 16 heads sharded 2-per-core across 8 cores (tensor parallel).
  - Latent path folded on the host: K = x @ (Wk_w @ Wc_w).T + (Wk_w @ Wc_b + Wk_b),
    same for V. Each core: three D=2048-contraction projections for its 256
    head-columns, causal attention for its 2 heads, partial output projection;
    partials summed on the host (allreduce-equivalent).
  - Activations feature-major ("transposed") on device; x transposed on host.
  - Softmax without max-subtraction (scores are O(1) by construction);
    denominator accumulated on DVE, partition-reduced with a ones-matmul,
    reciprocal broadcast via a DRAM bounce, applied to the attention output.
    Causal masking multiplies exp tiles on GPSIMD.
  - Matmul dtype float32r (TF32-like; measured ~125 ns per 128x128x512 MM).
"""

import contextlib
import math

import numpy as np

import concourse.bass as bass
import concourse.tile as tile
from concourse import bacc, mybir
from concourse.bass import ds, ts
from concourse.bass_utils import run_bass_kernel_spmd

P = 128
B, S, D = 2, 2048, 2048
H, DL = 16, 512
HD = D // H              # 128
NCORES = 8
HLOC = H // NCORES       # 2 heads per core
COL = HLOC * HD          # 256 per-core head columns
TOK = B * S              # 4096
TT = 512                 # token tile (projections, q tiles)
DCH = D // P             # 16 contraction chunks
SCALE = 1.0 / math.sqrt(HD)

DT = mybir.dt.float32r   # matmul/SBUF dtype
DTP = mybir.dt.float32   # PSUM dtype
F32 = mybir.dt.float32

_CACHE = {}


def _build(repeat=1, phases=(1, 1, 1), interleave=False, ATTN_L=5, mask_engine="mm", denom="dve", early_evict=True, x_bf16=True, out_bf16=True, use_bias=False):
    nc = bacc.Bacc("TRN2", target_bir_lowering=False, debug=False)

    xdt = mybir.dt.bfloat16 if x_bf16 else DT
    odt = mybir.dt.bfloat16 if out_bf16 else F32
    xT = nc.dram_tensor("xT", [B, S // TT, 4, P, 4, TT], xdt, kind="ExternalInput").ap()
    wq = nc.dram_tensor("wq", [D, COL], DT, kind="ExternalInput").ap()
    wk = nc.dram_tensor("wk", [D, COL], DT, kind="ExternalInput").ap()
    wv = nc.dram_tensor("wv", [D, COL], DT, kind="ExternalInput").ap()
    wo = nc.dram_tensor("wo", [COL, D], DT, kind="ExternalInput").ap()
    bq = nc.dram_tensor("bq", [P, HLOC], F32, kind="ExternalInput").ap()
    bk = nc.dram_tensor("bk", [P, HLOC], F32, kind="ExternalInput").ap()
    bv = nc.dram_tensor("bv", [P, HLOC], F32, kind="ExternalInput").ap()
    masks = nc.dram_tensor("masks", [4, P, TT], DT, kind="ExternalInput").ap()
    identd = nc.dram_tensor("ident", [P, P], DT, kind="ExternalInput").ap()
    onesd = nc.dram_tensor("ones", [P, 1], DT, kind="ExternalInput").ap()
    out = nc.dram_tensor("out", [TOK, D], odt, kind="ExternalOutput").ap()

    with tile.TileContext(nc) as tc:
        with (
            tc.tile_pool(name="const", bufs=1) as const,
            tc.tile_pool(name="wpool", bufs=1) as wpool,
            tc.tile_pool(name="xpool", bufs=4) as xpool,
            tc.tile_pool(name="actb", bufs=1) as actb,
            tc.tile_pool(name="expp", bufs=8) as expp,
            tc.tile_pool(name="daccp", bufs=(2 if denom != "pe" else 1)) as daccp,
            tc.tile_pool(name="stage", bufs=4) as stagep,
            tc.tile_pool(name="rp", bufs=2) as rp,
            tc.tile_pool(name="rbp", bufs=2) as rbp,
            tc.tile_pool(name="dram", bufs=4, space="DRAM") as dramp,
            tc.tile_pool(name="ps", bufs=8, space="PSUM") as psp,
        ):
            ident = const.tile([P, P], DT)
            nc.sync.dma_start(ident, identd)
            ones = const.tile([P, 1], DT)
            nc.sync.dma_start(ones, onesd)
            bq_sb = const.tile([P, HLOC], F32, tag="bq")
            nc.sync.dma_start(bq_sb, bq)
            bk_sb = const.tile([P, HLOC], F32, tag="bk")
            nc.sync.dma_start(bk_sb, bk)
            bv_sb = const.tile([P, HLOC], F32, tag="bv")
            nc.sync.dma_start(bv_sb, bv)

            wq_sb = wpool.tile([P, DCH, COL], DT, tag="wq")
            nc.sync.dma_start(wq_sb, wq.rearrange("(o p) c -> p o c", p=P))
            wk_sb = wpool.tile([P, DCH, COL], DT, tag="wk")
            nc.sync.dma_start(wk_sb, wk.rearrange("(o p) c -> p o c", p=P))
            wv_sb = wpool.tile([P, DCH, COL], DT, tag="wv")
            nc.sync.dma_start(wv_sb, wv.rearrange("(o p) c -> p o c", p=P))
            mask_sb = const.tile([P, 4, TT], DT)
            nc.sync.dma_start(mask_sb, masks.rearrange("m p q -> p m q"))
            wo_sb = wpool.tile([P, HLOC, D], DT, tag="wo")
            nc.sync.dma_start(wo_sb, wo.rearrange("(c p) d -> p c d", p=P))

            state = {"flip": 0}

            def emit_qkv(b, tok0, tt, qT_sb, kT_sb, v_sb):
                ps_acc = [
                    psp.tile([P, TT], DTP, tag="ps", name=f"ps_acc{j}")
                    for j in range(6)
                ]
                for quarter in range(4):
                    x_sb = xpool.tile([P, 4, TT], DT, tag="x", name="x_sb")
                    (nc.gpsimd if x_bf16 else nc.sync).dma_start(
                        x_sb, xT[b, tt, quarter]
                    )
                    for kq in range(4):
                        k = quarter * 4 + kq
                        for j, w_sb in enumerate(
                            (wq_sb, wq_sb, wk_sb, wk_sb, wv_sb, wv_sb)
                        ):
                            cc = j % 2
                            nc.tensor.matmul(
                                ps_acc[j],
                                w_sb[:, k, ts(cc, P)],
                                x_sb[:, kq, :],
                                start=(k == 0),
                                stop=(k == DCH - 1),
                            )
                vt_stages = []
                for cc in range(HLOC):
                    vt_stage = stagep.tile([P, TT], DT, tag="stage", name="vt_stage")
                    if use_bias:
                        nc.vector.tensor_scalar_add(
                            qT_sb[:, cc, ts(tt, TT)], ps_acc[0 + cc], bq_sb[:, ds(cc, 1)]
                        )
                        nc.vector.tensor_scalar_add(
                            kT_sb[:, cc, ts(tt, TT)], ps_acc[2 + cc], bk_sb[:, ds(cc, 1)]
                        )
                        nc.vector.tensor_scalar_add(
                            vt_stage, ps_acc[4 + cc], bv_sb[:, ds(cc, 1)]
                        )
                    else:
                        nc.vector.tensor_copy(qT_sb[:, cc, ts(tt, TT)], ps_acc[0 + cc])
                        nc.vector.tensor_copy(kT_sb[:, cc, ts(tt, TT)], ps_acc[2 + cc])
                        nc.vector.tensor_copy(vt_stage, ps_acc[4 + cc])
                    vt_stages.append((cc, vt_stage))

                def flush(cc, vt_stage, tt=tt):
                    for c4 in range(TT // P):
                        tp = psp.tile([P, P], DT, tag="ps", name="tp")
                        nc.tensor.transpose(tp, vt_stage[:, ts(c4, P)], ident)
                        nc.vector.tensor_copy(v_sb[:, tt * 4 + c4, ts(cc, P)], tp)

                return [lambda cc=cc, v=v: flush(cc, v) for cc, v in vt_stages]

            def emit_attn(qt, qT_sb, kT_sb, v_sb, o_sb, L=ATTN_L):
                nkt = (qt + 1) * (TT // P)
                o_ps = {}
                d_ps = {}
                e_tiles = {}
                for h in range(HLOC):
                    o_ps[h] = psp.tile([P, TT], DTP, tag="ps", name="o_ps")
                    d_ps[h] = psp.tile([P, TT], DTP, tag="ps", name="d_ps")[ds(0, 1), :]

                def emit_score(h, kt):
                    s_ps = psp.tile([P, TT], DTP, tag="ps", name="s_ps")
                    m = kt - qt * (TT // P)
                    diag = 0 <= m < 4
                    if diag:
                        nc.tensor.matmul(
                            s_ps, ident, mask_sb[:, m, :], start=True, stop=False
                        )
                    nc.tensor.matmul(
                        s_ps,
                        kT_sb[:, h, ts(kt, P)],
                        qT_sb[:, h, ts(qt, TT)],
                        start=not diag,
                        stop=True,
                    )
                    e_sb = expp.tile([P, TT], DT, tag="exp", name="e_sb")
                    nc.scalar.activation(
                        e_sb, s_ps, mybir.ActivationFunctionType.Exp, scale=SCALE
                    )
                    e_tiles[(h, kt)] = e_sb

                def emit_consume(h, kt):
                    e_sb = e_tiles.pop((h, kt))
                    nc.tensor.matmul(
                        d_ps[h], ones, e_sb,
                        start=(kt == 0), stop=(kt == nkt - 1),
                    )
                    nc.tensor.matmul(
                        o_ps[h],
                        v_sb[:, kt, ts(h, P)],
                        e_sb,
                        start=(kt == 0),
                        stop=(kt == nkt - 1),
                    )

                # software pipeline: scores run L blocks ahead of consumers
                for kt in range(min(L, nkt)):
                    for h in range(HLOC):
                        emit_score(h, kt)
                for kt in range(nkt):
                    for h in range(HLOC):
                        emit_consume(h, kt)
                        if kt + L < nkt:
                            emit_score(h, kt + L)

                for h in range(HLOC):
                    r_seg = rp.tile([1, TT], F32, tag="r", name="r_seg")
                    nc.vector.reciprocal(r_seg, d_ps[h])
                    r_dram = dramp.tile([1, TT], F32, name="r_dram")
                    nc.sync.dma_start(r_dram, r_seg)
                    rb_sb = rbp.tile([P, TT], F32, tag="rb", name="rb_sb")
                    nc.sync.dma_start(rb_sb, r_dram.to_broadcast((P, TT)))
                    nc.vector.tensor_copy(o_sb[:, h, ts(qt, TT)], o_ps[h])
                    nc.vector.tensor_tensor(
                        o_sb[:, h, ts(qt, TT)], o_sb[:, h, ts(qt, TT)], rb_sb,
                        mybir.AluOpType.mult
                    )

            def emit_proj(tok0, qt, o_sb):
                for tc4 in range(TT // P):
                    tch = qt * 4 + tc4
                    for nt in range(D // TT):
                        p_ps = psp.tile([P, TT], DTP, tag="ps", name="p_ps")
                        for h in range(HLOC):
                            nc.tensor.matmul(
                                p_ps,
                                o_sb[:, h, ts(tch, P)],
                                wo_sb[:, h, ts(nt, TT)],
                                start=(h == 0),
                                stop=(h == HLOC - 1),
                            )
                        o_stage = stagep.tile([P, TT], odt, tag="ostage", name="o_stage")
                        if state["flip"] % 4 != 3:
                            nc.vector.tensor_copy(o_stage, p_ps)
                        else:
                            nc.scalar.activation(
                                o_stage, p_ps, mybir.ActivationFunctionType.Copy
                            )
                        state["flip"] += 1
                        nc.sync.dma_start(
                            out[ds(tok0 + tch * P, P), ts(nt, TT)], o_stage
                        )

            rep_ctx = tc.For_i(0, repeat, 1) if repeat > 1 else contextlib.nullcontext()
            with rep_ctx:
                for b in range(B):
                    tok0 = b * S
                    qT_sb = actb.tile([P, HLOC, S], DT, tag="qT", name="qT_sb")
                    kT_sb = actb.tile([P, HLOC, S], DT, tag="kT", name="kT_sb")
                    v_sb = actb.tile([P, S // P, COL], DT, tag="v", name="v_sb")
                    o_sb = actb.tile([P, HLOC, S], DT, tag="o", name="o_sb")

                    pending_tp = []
                    if phases[0]:
                        for tt in range(S // TT):
                            tps = emit_qkv(b, tok0, tt, qT_sb, kT_sb, v_sb)
                            for f in pending_tp:
                                f()
                            pending_tp = tps
                        for f in pending_tp:
                            f()
                    prev_qt = None
                    for qt in range(S // TT):
                        if phases[1]:
                            emit_attn(qt, qT_sb, kT_sb, v_sb, o_sb)
                        if phases[2] and prev_qt is not None:
                            emit_proj(tok0, prev_qt, o_sb)
                        prev_qt = qt
                    if phases[2] and prev_qt is not None:
                        emit_proj(tok0, prev_qt, o_sb)
    nc.compile()
    return nc


def _prep_inputs(x, Wq_w, Wq_b, Wc_w, Wc_b, Wk_w, Wk_b, Wv_w, Wv_b, Wo_w, Wo_b, x_bf16=True, mask_mode="mm"):
    import ml_dtypes
    f32 = np.float32
    x = np.ascontiguousarray(np.asarray(x, f32).reshape(TOK, D))
    xT = np.ascontiguousarray(x.T)
    # pre-arranged contiguous blocks: [B, S//TT, 2, 128, 8, TT]
    # xprep[b, tt, h, p, i, t] = xT[(h*8+i)*128 + p, b*S + tt*TT + t]
    xp = xT.reshape(4, 4, P, B, S // TT, TT)         # [q, i, p, b, tt, t]
    xprep = np.ascontiguousarray(xp.transpose(3, 4, 0, 2, 1, 5))
    if x_bf16:
        xprep = xprep.astype(ml_dtypes.bfloat16)
    else:
        xprep = xprep.astype(f32)
    Wk_eff = np.asarray(Wk_w, f32) @ np.asarray(Wc_w, f32)     # [D, D]
    Wv_eff = np.asarray(Wv_w, f32) @ np.asarray(Wc_w, f32)
    bk_eff = np.asarray(Wk_w, f32) @ np.asarray(Wc_b, f32) + np.asarray(Wk_b, f32)
    bv_eff = np.asarray(Wv_w, f32) @ np.asarray(Wc_b, f32) + np.asarray(Wv_b, f32)

    keep = (np.arange(P)[None, :, None] + P * np.arange(4)[:, None, None]
            <= np.arange(TT)[None, None, :])                   # [4, 128, 512]
    if mask_mode == "mm":
        m = np.where(keep, 0.0, -340.0).astype(f32)
    else:
        m = keep.astype(f32)

    in_maps = []
    for c in range(NCORES):
        cols = slice(c * COL, (c + 1) * COL)
        in_maps.append({
            "xT": xprep,
            "wq": np.ascontiguousarray(np.asarray(Wq_w, f32)[cols, :].T),
            "wk": np.ascontiguousarray(Wk_eff[cols, :].T),
            "wv": np.ascontiguousarray(Wv_eff[cols, :].T),
            "wo": np.ascontiguousarray(np.asarray(Wo_w, f32)[:, cols].T),
            "bq": np.ascontiguousarray(np.asarray(Wq_b, f32)[cols].reshape(HLOC, P).T),
            "bk": np.ascontiguousarray(bk_eff[cols].reshape(HLOC, P).T),
            "bv": np.ascontiguousarray(bv_eff[cols].reshape(HLOC, P).T),
            "masks": m,
            "ident": np.eye(P, dtype=f32),
            "ones": np.ones((P, 1), f32),
        })
    return in_maps


def kernel(**inputs):
    use_bias = any(
        np.any(np.asarray(inputs[k])) for k in ("Wq_b", "Wc_b", "Wk_b", "Wv_b")
    )
    key = ("nc", bool(use_bias))
    if key not in _CACHE:
        _CACHE[key] = _build(use_bias=use_bias)
    nc = _CACHE[key]
    in_maps = _prep_inputs(**inputs)
    res = run_bass_kernel_spmd(nc, in_maps, core_ids=list(range(NCORES)))
    acc = res.results[0]["out"].astype(np.float32)
    for c in range(1, NCORES):
        acc = acc + res.results[c]["out"]
    acc = acc + np.asarray(inputs["Wo_b"], np.float32)[None, :]
    return acc.reshape(B, S, D)
